# revision 4
# baseline (speedup 1.0000x reference)
"""Multi-head attention kernel for Trainium2, 8 NeuronCores.

Problem: B=4, T=2048, D=1024, 16 heads x 64 head-dim, key-padding mask.
Sharding: core = (batch b, head-half hh); each of the 8 cores computes one
batch's attention over 8 heads (512 channels) and a partial output
projection; the host sums the two partials per batch and adds the bias.

Device-side layout is channel-major throughout:
  hT   = hidden[b]^T                      [D, T]    (bf16)
  Q^T  = (Wq_half)^T hT                   [C, T]    via lhsT=Wq, rhs=hT
  K^T/V computed only at unmasked key positions (host compacts keys using
  the attention mask, zero-padded to TKV).
  S^T  = K^T-block^T Q^T                  [t2, t1]  (PSUM, per 128-key block)
  P^T  = exp(scale*S^T + bias[t2])        (ACT, bias=-1e9 at padded keys)
  O^T  += V_aug-block^T P^T               [65, t1]  (PSUM accum; row 64 = ones
         column of V_aug => softmax denominator for free)
  out  = (O^T/denom)^T-concat @ Wo_half   [T, D]    (partial, fp32)
"""

import numpy as np
import ml_dtypes

import concourse.bacc as bacc
import concourse.tile as tile
from concourse import mybir
from concourse.bass_utils import run_bass_kernel_spmd

BF16 = mybir.dt.bfloat16
F32 = mybir.dt.float32

B, T, D = 4, 2048, 1024
HEADS, DH = 16, 64
C = 512              # channels per core (8 heads)
SCALE = DH ** -0.5
NEG = -1.0e9

TKV = 1280           # compacted+padded key count (seed-0 max unmasked is 1075)
KB = D // 128        # 8 contraction blocks for the projections
NBLK = TKV // 128    # 10 key blocks


def build_bass(tkv=TKV):
    nblk = tkv // 128
    nc = bacc.Bacc("TRN2", target_bir_lowering=False)

    hT_d = nc.dram_tensor("hT", [D, T], BF16, kind="ExternalInput")
    hTkv_d = nc.dram_tensor("hTkv", [D, tkv], BF16, kind="ExternalInput")
    wq_d = nc.dram_tensor("wq", [D, C], BF16, kind="ExternalInput")
    wk_d = nc.dram_tensor("wk", [D, C], BF16, kind="ExternalInput")
    wv_d = nc.dram_tensor("wv", [D, C], BF16, kind="ExternalInput")
    wo_d = nc.dram_tensor("wo", [C, D], BF16, kind="ExternalInput")
    mb_d = nc.dram_tensor("mb", [128, nblk], F32, kind="ExternalInput")
    out_d = nc.dram_tensor("out", [T, D], F32, kind="ExternalOutput")

    with tile.TileContext(nc) as tc:
        with (
            tc.tile_pool(name="persist", bufs=1) as persist,
            tc.tile_pool(name="pt", bufs=4) as ppool,
            tc.tile_pool(name="recip", bufs=4) as rpool,
            tc.tile_pool(name="bcast", bufs=4) as bpool,
            tc.tile_pool(name="ostage", bufs=4) as ostage,
        ):
            # ---- persistent SBUF tensors + input DMA ----
            hT = persist.tile([128, KB, T], BF16)
            hTkv = persist.tile([128, KB, tkv], BF16)
            wq = persist.tile([128, KB, C], BF16)
            wk = persist.tile([128, KB, C], BF16)
            wv = persist.tile([128, KB, C], BF16)
            wo = persist.tile([128, C // 128, D], BF16)
            mb = persist.tile([128, nblk], F32)
            qT = persist.tile([128, C // 128, T], BF16)
            kT = persist.tile([128, C // 128, tkv], BF16)
            vaug = persist.tile([128, nblk, 8 * 65], BF16)
            ocatT = persist.tile([128, C // 128, T], BF16)
            ones64 = persist.tile([1, 64], F32)

            nc.sync.dma_start(wq[:], wq_d.ap().rearrange("(k p) c -> p k c", p=128))
            nc.sync.dma_start(hT[:], hT_d.ap().rearrange("(k p) t -> p k t", p=128))
            nc.sync.dma_start(hTkv[:], hTkv_d.ap().rearrange("(k p) t -> p k t", p=128))
            nc.sync.dma_start(wk[:], wk_d.ap().rearrange("(k p) c -> p k c", p=128))
            nc.sync.dma_start(wv[:], wv_d.ap().rearrange("(k p) c -> p k c", p=128))
            nc.sync.dma_start(wo[:], wo_d.ap().rearrange("(k p) e -> p k e", p=128))
            nc.sync.dma_start(mb[:], mb_d.ap())
            nc.vector.memset(ones64[:], 1.0)
            # ones column of V_aug (slot 64 of each head's 65-wide group)
            va4 = vaug[:].rearrange("p n (h w) -> p n h w", w=65)
            nc.vector.memset(va4[:, :, :, 64:65], 1.0)

            # ---- phase 1: QKV projections ----
            with tc.tile_pool(name="qkv_ps", bufs=4, space="PSUM") as qps:
                # Q^T [C, T]: lhsT = wq [D, C], rhs = hT [D, T]
                for cb in range(C // 128):
                    for tch in range(T // 512):
                        ps = qps.tile([128, 512], F32, tag="qkv")
                        for k in range(KB):
                            nc.tensor.matmul(
                                ps[:],
                                wq[:, k, cb * 128:(cb + 1) * 128],
                                hT[:, k, tch * 512:(tch + 1) * 512],
                                start=(k == 0), stop=(k == KB - 1),
                            )
                        nc.vector.tensor_copy(qT[:, cb, tch * 512:(tch + 1) * 512], ps[:])
                # K^T [C, tkv]
                for cb in range(C // 128):
                    for t0 in range(0, tkv, 512):
                        w = min(512, tkv - t0)
                        ps = qps.tile([128, 512], F32, tag="qkv")
                        for k in range(KB):
                            nc.tensor.matmul(
                                ps[:, :w],
                                wk[:, k, cb * 128:(cb + 1) * 128],
                                hTkv[:, k, t0:t0 + w],
                                start=(k == 0), stop=(k == KB - 1),
                            )
                        nc.vector.tensor_copy(kT[:, cb, t0:t0 + w], ps[:, :w])
                # V [tkv, C] -> packed into vaug (65-wide per head)
                for tb in range(nblk):
                    ps = qps.tile([128, 512], F32, tag="qkv")
                    for k in range(KB):
                        nc.tensor.matmul(
                            ps[:],
                            hTkv[:, k, tb * 128:(tb + 1) * 128],
                            wv[:, k, :],
                            start=(k == 0), stop=(k == KB - 1),
                        )
                    src = ps[:].rearrange("p (h w) -> p h w", w=64)
                    nc.vector.tensor_copy(va4[:, tb, :, 0:64], src[:])

            # ---- phase 2: attention, per head pair (cb) and t1 half ----
            with (
                tc.tile_pool(name="s_ps", bufs=2, space="PSUM") as spool,
                tc.tile_pool(name="o_ps", bufs=4, space="PSUM") as opool,
            ):
                for cb in range(C // 128):
                    for half in range(2):
                        t1o = half * 1024
                        ops = [opool.tile([65, 512], F32, tag="o", name=f"o_{cb}_{half}_{i}")
                               for i in range(4)]
                        for blk in range(nblk):
                            pts = []
                            for hh in range(2):  # head within pair
                                p0 = hh * 64
                                s = spool.tile([128, 1024], F32, tag="s")
                                for c2 in range(2):
                                    nc.tensor.matmul(
                                        s[:, c2 * 512:(c2 + 1) * 512],
                                        kT[p0:p0 + 64, cb, blk * 128:(blk + 1) * 128],
                                        qT[p0:p0 + 64, cb, t1o + c2 * 512:t1o + (c2 + 1) * 512],
                                        start=True, stop=True,
                                    )
                                pt = ppool.tile([128, 1024], BF16, tag="pt")
                                nc.scalar.activation(
                                    pt[:], s[:],
                                    mybir.ActivationFunctionType.Exp,
                                    bias=mb[:, blk:blk + 1], scale=SCALE,
                                )
                                pts.append(pt)
                            for hh in range(2):
                                h65 = (2 * cb + hh) * 65
                                for c2 in range(2):
                                    nc.tensor.matmul(
                                        ops[2 * hh + c2][:],
                                        vaug[:, blk, h65:h65 + 65],
                                        pts[hh][:, c2 * 512:(c2 + 1) * 512],
                                        start=(blk == 0), stop=(blk == nblk - 1),
                                    )
                        # normalize: ocatT[head rows, t1] = O / denom
                        for hh in range(2):
                            for c2 in range(2):
                                o = ops[2 * hh + c2]
                                rc = rpool.tile([1, 512], F32, tag="rc")
                                nc.vector.reciprocal(rc[:], o[64:65, :])
                                bc = spool.tile([64, 512], F32, tag="s")
                                nc.tensor.matmul(bc[:], ones64[:], rc[:],
                                                 start=True, stop=True)
                                bcs = bpool.tile([64, 512], F32, tag="bcs")
                                nc.vector.tensor_copy(bcs[:], bc[:])
                                nc.vector.tensor_mul(
                                    ocatT[hh * 64:(hh + 1) * 64, cb,
                                          t1o + c2 * 512:t1o + (c2 + 1) * 512],
                                    o[0:64, :], bcs[:],
                                )

            # ---- phase 3: output projection (partial; host adds pair + bias) ----
            with tc.tile_pool(name="proj_ps", bufs=4, space="PSUM") as jps:
                for tt in range(T // 128):
                    for e in range(D // 512):
                        ps = jps.tile([128, 512], F32, tag="proj")
                        for cb in range(C // 128):
                            nc.tensor.matmul(
                                ps[:],
                                ocatT[:, cb, tt * 128:(tt + 1) * 128],
                                wo[:, cb, e * 512:(e + 1) * 512],
                                start=(cb == 0), stop=(cb == C // 128 - 1),
                            )
                        ot = ostage.tile([128, 512], F32, tag="os")
                        nc.vector.tensor_copy(ot[:], ps[:])
                        nc.sync.dma_start(
                            out_d.ap()[tt * 128:(tt + 1) * 128, e * 512:(e + 1) * 512],
                            ot[:],
                        )

    nc.compile()
    return nc


_NC_CACHE = {}


def _get_nc(tkv=TKV):
    if tkv not in _NC_CACHE:
        _NC_CACHE[tkv] = build_bass(tkv)
    return _NC_CACHE[tkv]


def make_in_maps(inputs, tkv=TKV):
    hidden = np.asarray(inputs["hidden_states"], np.float32)
    mask = np.asarray(inputs["attention_mask"])
    Wq = np.asarray(inputs["Wq"], np.float32).astype(ml_dtypes.bfloat16)
    Wk = np.asarray(inputs["Wk"], np.float32).astype(ml_dtypes.bfloat16)
    Wv = np.asarray(inputs["Wv"], np.float32).astype(ml_dtypes.bfloat16)
    Wo = np.asarray(inputs["Wo"], np.float32).astype(ml_dtypes.bfloat16)

    nblk = tkv // 128
    in_maps = []
    for core in range(8):
        b, hh = divmod(core, 2)
        ch = slice(hh * C, (hh + 1) * C)
        hTb = np.ascontiguousarray(hidden[b].T).astype(ml_dtypes.bfloat16)
        idx = np.nonzero(mask[b])[0]
        n = len(idx)
        assert n <= tkv, f"unmasked keys {n} > TKV {tkv}"
        hTkv = np.zeros((D, tkv), ml_dtypes.bfloat16)
        hTkv[:, :n] = hidden[b].T[:, idx].astype(ml_dtypes.bfloat16)
        mb = np.full(tkv, NEG, np.float32)
        mb[:n] = 0.0
        mb = np.ascontiguousarray(mb.reshape(nblk, 128).T)  # [128, nblk]
        in_maps.append({
            "hT": hTb,
            "hTkv": hTkv,
            "wq": np.ascontiguousarray(Wq[:, ch]),
            "wk": np.ascontiguousarray(Wk[:, ch]),
            "wv": np.ascontiguousarray(Wv[:, ch]),
            "wo": np.ascontiguousarray(Wo[ch, :]),
            "mb": mb,
        })
    return in_maps


def gather(results, inputs):
    bo = np.asarray(inputs["bo"], np.float32)
    out = np.empty((B, T, D), np.float32)
    for b in range(B):
        out[b] = results[2 * b]["out"] + results[2 * b + 1]["out"] + bo
    return out


def _run(inputs, trace=False):
    nc = _get_nc()
    in_maps = make_in_maps(inputs)
    res = run_bass_kernel_spmd(nc, in_maps, core_ids=list(range(8)), trace=trace)
    return gather(res.results, inputs), res


def kernel(**inputs):
    out, _ = _run(inputs)
    return out


# revision 6
# speedup vs baseline: 1.1988x; 1.1988x over previous
"""Multi-head attention kernel for Trainium2, 8 NeuronCores.

Problem: B=4, T=2048, D=1024, 16 heads x 64 head-dim, key-padding mask.
Sharding: core = (batch b, head-half hh); each of the 8 cores computes one
batch's attention over 8 heads (512 channels) and a partial output
projection; the host sums the two partials per batch and adds the bias.

Device-side layout is channel-major throughout:
  hT   = hidden[b]^T                      [D, T]    (bf16)
  Q^T  = (Wq_half)^T hT                   [C, T]    via lhsT=Wq, rhs=hT
  K^T/V computed only at unmasked key positions (host compacts keys using
  the attention mask, zero-padded to TKV).
  S^T  = K^T-block^T Q^T                  [t2, t1]  PSUM; the two heads of a
         pair are row-tiled (K=64 at partition bases 0/64) so their matmuls
         overlap in the PE array.
  P^T  = exp(scale*S^T + bias[t2])        (ACT, bias=-1e9 at padded keys)
  O    pair-accumulated col-tiled: head A -> PSUM rows 0:64, head B -> rows
       64:128 of one bank; a parallel rowsum bank accumulates ones^T P^T
       replicated across 64 rows per head => denominator aligned with O.
  out  = (O/denom) concat @ Wo_half       [T, D]    (partial, fp32)
"""

import numpy as np
import ml_dtypes

import concourse.bacc as bacc
import concourse.tile as tile
from concourse import mybir
from concourse.bass_utils import run_bass_kernel_spmd

BF16 = mybir.dt.bfloat16
F32 = mybir.dt.float32

B, T, D = 4, 2048, 1024
HEADS, DH = 16, 64
C = 512              # channels per core (8 heads)
SCALE = DH ** -0.5
NEG = -1.0e9

TKV = 1152           # compacted+padded key count (seed-0 max unmasked is 1075)
KB = D // 128        # 8 contraction blocks for the projections


def build_bass(tkv=TKV):
    nblk = tkv // 128
    nc = bacc.Bacc("TRN2", target_bir_lowering=False)

    hT_d = nc.dram_tensor("hT", [D, T], BF16, kind="ExternalInput")
    hTkv_d = nc.dram_tensor("hTkv", [D, tkv], BF16, kind="ExternalInput")
    wq_d = nc.dram_tensor("wq", [D, C], BF16, kind="ExternalInput")
    wk_d = nc.dram_tensor("wk", [D, C], BF16, kind="ExternalInput")
    wv_d = nc.dram_tensor("wv", [D, C], BF16, kind="ExternalInput")
    wo_d = nc.dram_tensor("wo", [C, D], BF16, kind="ExternalInput")
    mb_d = nc.dram_tensor("mb", [128, nblk], F32, kind="ExternalInput")
    out_d = nc.dram_tensor("out", [T, D], F32, kind="ExternalOutput")

    with tile.TileContext(nc) as tc:
        with (
            tc.tile_pool(name="persist", bufs=1) as persist,
            tc.tile_pool(name="pt", bufs=4) as ppool,
            tc.tile_pool(name="rs", bufs=4) as rpool,
            tc.tile_pool(name="ostage", bufs=4) as ostage,
        ):
            # ---- persistent SBUF tensors + input DMA ----
            hT = persist.tile([128, KB, T], BF16)
            hTkv = persist.tile([128, KB, tkv], BF16)
            wq = persist.tile([128, KB, C], BF16)
            wk = persist.tile([128, KB, C], BF16)
            wv = persist.tile([128, KB, C], BF16)
            wo = persist.tile([128, C // 128, D], BF16)
            mb = persist.tile([128, nblk], F32)
            qT = persist.tile([128, C // 128, T], BF16)
            kT = persist.tile([128, C // 128, tkv], BF16)
            vsb = persist.tile([128, nblk, C], BF16)
            ocatT = persist.tile([128, C // 128, T], BF16)
            ones64 = persist.tile([128, 64], BF16)

            nc.sync.dma_start(wq[:], wq_d.ap().rearrange("(k p) c -> p k c", p=128))
            nc.sync.dma_start(hT[:], hT_d.ap().rearrange("(k p) t -> p k t", p=128))
            nc.sync.dma_start(hTkv[:], hTkv_d.ap().rearrange("(k p) t -> p k t", p=128))
            nc.sync.dma_start(wk[:], wk_d.ap().rearrange("(k p) c -> p k c", p=128))
            nc.sync.dma_start(wv[:], wv_d.ap().rearrange("(k p) c -> p k c", p=128))
            nc.sync.dma_start(wo[:], wo_d.ap().rearrange("(k p) e -> p k e", p=128))
            nc.sync.dma_start(mb[:], mb_d.ap())
            nc.vector.memset(ones64[:], 1.0)

            # ---- phase 1: QKV projections ----
            with tc.tile_pool(name="qkv_ps", bufs=4, space="PSUM") as qps:
                # Q^T [C, T]: lhsT = wq [D, C], rhs = hT [D, T]
                for cb in range(C // 128):
                    for tch in range(T // 512):
                        ps = qps.tile([128, 512], F32, tag="qkv")
                        for k in range(KB):
                            nc.tensor.matmul(
                                ps[:],
                                wq[:, k, cb * 128:(cb + 1) * 128],
                                hT[:, k, tch * 512:(tch + 1) * 512],
                                start=(k == 0), stop=(k == KB - 1),
                            )
                        nc.vector.tensor_copy(qT[:, cb, tch * 512:(tch + 1) * 512], ps[:])
                # K^T [C, tkv]
                for cb in range(C // 128):
                    for t0 in range(0, tkv, 512):
                        w = min(512, tkv - t0)
                        ps = qps.tile([128, 512], F32, tag="qkv")
                        for k in range(KB):
                            nc.tensor.matmul(
                                ps[:, :w],
                                wk[:, k, cb * 128:(cb + 1) * 128],
                                hTkv[:, k, t0:t0 + w],
                                start=(k == 0), stop=(k == KB - 1),
                            )
                        nc.vector.tensor_copy(kT[:, cb, t0:t0 + w], ps[:, :w])
                # V [tkv, C]
                for tb in range(nblk):
                    ps = qps.tile([128, 512], F32, tag="qkv")
                    for k in range(KB):
                        nc.tensor.matmul(
                            ps[:],
                            hTkv[:, k, tb * 128:(tb + 1) * 128],
                            wv[:, k, :],
                            start=(k == 0), stop=(k == KB - 1),
                        )
                    nc.vector.tensor_copy(vsb[:, tb, :], ps[:])

            # ---- phase 2: attention, per head pair (cb) and t1 half ----
            with (
                tc.tile_pool(name="s_ps", bufs=2, space="PSUM") as spool,
                tc.tile_pool(name="o_ps", bufs=2, space="PSUM") as opool,
                tc.tile_pool(name="r_ps", bufs=2, space="PSUM") as rps,
            ):
                for cb in range(C // 128):
                    hA, hB = 2 * cb, 2 * cb + 1
                    for half in range(2):
                        t1o = half * 1024
                        ops = [opool.tile([128, 512], F32, tag="o", name=f"o_{cb}_{half}_{i}")
                               for i in range(2)]
                        rss = [rps.tile([128, 512], F32, tag="r", name=f"r_{cb}_{half}_{i}")
                               for i in range(2)]
                        for blk in range(nblk):
                            sa = spool.tile([128, 1024], F32, tag="s")
                            sb_t = spool.tile([128, 1024], F32, tag="s")
                            for c2 in range(2):
                                t1s = slice(t1o + c2 * 512, t1o + (c2 + 1) * 512)
                                nc.tensor.matmul(
                                    sa[:, c2 * 512:(c2 + 1) * 512],
                                    kT[0:64, cb, blk * 128:(blk + 1) * 128],
                                    qT[0:64, cb, t1s],
                                    start=True, stop=True,
                                )
                                nc.tensor.matmul(
                                    sb_t[:, c2 * 512:(c2 + 1) * 512],
                                    kT[64:128, cb, blk * 128:(blk + 1) * 128],
                                    qT[64:128, cb, t1s],
                                    start=True, stop=True,
                                )
                            pta = ppool.tile([128, 1024], BF16, tag="pt")
                            nc.scalar.activation(
                                pta[:], sa[:], mybir.ActivationFunctionType.Exp,
                                bias=mb[:, blk:blk + 1], scale=SCALE,
                            )
                            ptb = ppool.tile([128, 1024], BF16, tag="pt")
                            nc.scalar.activation(
                                ptb[:], sb_t[:], mybir.ActivationFunctionType.Exp,
                                bias=mb[:, blk:blk + 1], scale=SCALE,
                            )
                            st, sp = (blk == 0), (blk == nblk - 1)
                            for c2 in range(2):
                                c2s = slice(c2 * 512, (c2 + 1) * 512)
                                # col-tiled pair: head A -> rows 0:64, head B -> rows 64:128
                                # of one bank; pending-zero is per-partition, so each half
                                # starts its own accumulation group at blk 0.
                                nc.tensor.matmul(
                                    ops[c2][0:64, :], vsb[:, blk, hA * 64:hA * 64 + 64],
                                    pta[:, c2s], start=st, stop=sp,
                                    tile_position=(0, 0), skip_group_check=True,
                                )
                                nc.tensor.matmul(
                                    ops[c2][64:128, :], vsb[:, blk, hB * 64:hB * 64 + 64],
                                    ptb[:, c2s], start=st, stop=sp,
                                    tile_position=(0, 64), skip_group_check=True,
                                )
                                nc.tensor.matmul(
                                    rss[c2][0:64, :], ones64[:],
                                    pta[:, c2s], start=st, stop=sp,
                                    tile_position=(0, 0), skip_group_check=True,
                                )
                                nc.tensor.matmul(
                                    rss[c2][64:128, :], ones64[:],
                                    ptb[:, c2s], start=st, stop=sp,
                                    tile_position=(0, 64), skip_group_check=True,
                                )
                        # normalize both heads at once: denom replicas are partition-aligned
                        for c2 in range(2):
                            t1s = slice(t1o + c2 * 512, t1o + (c2 + 1) * 512)
                            rsb = rpool.tile([128, 512], F32, tag="rs")
                            nc.vector.reciprocal(rsb[:], rss[c2][:])
                            nc.vector.tensor_mul(ocatT[:, cb, t1s], ops[c2][:], rsb[:])

            # ---- phase 3: output projection (partial; host adds pair + bias) ----
            with tc.tile_pool(name="proj_ps", bufs=4, space="PSUM") as jps:
                for tt in range(T // 128):
                    for e in range(D // 512):
                        ps = jps.tile([128, 512], F32, tag="proj")
                        for cb in range(C // 128):
                            nc.tensor.matmul(
                                ps[:],
                                ocatT[:, cb, tt * 128:(tt + 1) * 128],
                                wo[:, cb, e * 512:(e + 1) * 512],
                                start=(cb == 0), stop=(cb == C // 128 - 1),
                            )
                        ot = ostage.tile([128, 512], F32, tag="os")
                        nc.vector.tensor_copy(ot[:], ps[:])
                        nc.sync.dma_start(
                            out_d.ap()[tt * 128:(tt + 1) * 128, e * 512:(e + 1) * 512],
                            ot[:],
                        )

    nc.compile()
    return nc


_NC_CACHE = {}


def _get_nc(tkv=TKV):
    if tkv not in _NC_CACHE:
        _NC_CACHE[tkv] = build_bass(tkv)
    return _NC_CACHE[tkv]


def make_in_maps(inputs, tkv=TKV):
    hidden = np.asarray(inputs["hidden_states"], np.float32)
    mask = np.asarray(inputs["attention_mask"])
    Wq = np.asarray(inputs["Wq"], np.float32).astype(ml_dtypes.bfloat16)
    Wk = np.asarray(inputs["Wk"], np.float32).astype(ml_dtypes.bfloat16)
    Wv = np.asarray(inputs["Wv"], np.float32).astype(ml_dtypes.bfloat16)
    Wo = np.asarray(inputs["Wo"], np.float32).astype(ml_dtypes.bfloat16)

    nblk = tkv // 128
    in_maps = []
    for core in range(8):
        b, hh = divmod(core, 2)
        ch = slice(hh * C, (hh + 1) * C)
        hTb = np.ascontiguousarray(hidden[b].T).astype(ml_dtypes.bfloat16)
        idx = np.nonzero(mask[b])[0]
        n = len(idx)
        assert n <= tkv, f"unmasked keys {n} > TKV {tkv}"
        hTkv = np.zeros((D, tkv), ml_dtypes.bfloat16)
        hTkv[:, :n] = hidden[b].T[:, idx].astype(ml_dtypes.bfloat16)
        mbv = np.full(tkv, NEG, np.float32)
        mbv[:n] = 0.0
        mbv = np.ascontiguousarray(mbv.reshape(nblk, 128).T)  # [128, nblk]
        in_maps.append({
            "hT": hTb,
            "hTkv": hTkv,
            "wq": np.ascontiguousarray(Wq[:, ch]),
            "wk": np.ascontiguousarray(Wk[:, ch]),
            "wv": np.ascontiguousarray(Wv[:, ch]),
            "wo": np.ascontiguousarray(Wo[ch, :]),
            "mb": mbv,
        })
    return in_maps


def gather(results, inputs):
    bo = np.asarray(inputs["bo"], np.float32)
    out = np.empty((B, T, D), np.float32)
    for b in range(B):
        out[b] = results[2 * b]["out"] + results[2 * b + 1]["out"] + bo
    return out


def _pick_tkv(inputs):
    mask = np.asarray(inputs["attention_mask"])
    nmax = int(mask.sum(axis=1).max())
    return max(TKV, -(-nmax // 128) * 128)


def _run(inputs, trace=False):
    tkv = _pick_tkv(inputs)
    nc = _get_nc(tkv)
    in_maps = make_in_maps(inputs, tkv)
    res = run_bass_kernel_spmd(nc, in_maps, core_ids=list(range(8)), trace=trace)
    return gather(res.results, inputs), res


def kernel(**inputs):
    out, _ = _run(inputs)
    return out


# revision 7
# speedup vs baseline: 1.3687x; 1.1417x over previous
"""Multi-head attention kernel for Trainium2, 8 NeuronCores.

Problem: B=4, T=2048, D=1024, 16 heads x 64 head-dim, key-padding mask.
Sharding: core = (batch b, head-half hh); each of the 8 cores computes one
batch's attention over 8 heads (512 channels) and a partial output
projection; the host sums the two partials per batch and adds the bias.

Device-side layout is channel-major throughout:
  hT   = hidden[b]^T                      [D, T]    (bf16)
  Q^T  = (Wq_half)^T hT                   [C, T]    via lhsT=Wq, rhs=hT
  K^T/V computed only at unmasked key positions (host compacts keys using
  the attention mask, zero-padded to TKV).
  S^T  = K^T-block^T Q^T                  [t2, t1]  PSUM; the two heads of a
         pair are row-tiled (K=64 at partition bases 0/64) so their matmuls
         overlap in the PE array.
  P^T  = exp(scale*S^T + bias[t2])        (ACT, bias=-1e9 at padded keys)
  O    pair-accumulated col-tiled: head A -> PSUM rows 0:64, head B -> rows
       64:128 of one bank; a parallel rowsum bank accumulates ones^T P^T
       replicated across 64 rows per head => denominator aligned with O.
  out  = (O/denom) concat @ Wo_half       [T, D]    (partial, fp32)
"""

import numpy as np
import ml_dtypes

import concourse.bacc as bacc
import concourse.tile as tile
from concourse import mybir
from concourse.bass_utils import run_bass_kernel_spmd

BF16 = mybir.dt.bfloat16
F32 = mybir.dt.float32

B, T, D = 4, 2048, 1024
HEADS, DH = 16, 64
C = 512              # channels per core (8 heads)
SCALE = DH ** -0.5
NEG = -1.0e9

TKV = 1152           # compacted+padded key count (seed-0 max unmasked is 1075)
KB = D // 128        # 8 contraction blocks for the projections


def build_bass(tkv=TKV):
    nblk = tkv // 128
    nc = bacc.Bacc("TRN2", target_bir_lowering=False)

    hT_d = nc.dram_tensor("hT", [D, T], BF16, kind="ExternalInput")
    hTkv_d = nc.dram_tensor("hTkv", [D, tkv], BF16, kind="ExternalInput")
    wq_d = nc.dram_tensor("wq", [D, C], BF16, kind="ExternalInput")
    wk_d = nc.dram_tensor("wk", [D, C], BF16, kind="ExternalInput")
    wv_d = nc.dram_tensor("wv", [D, C], BF16, kind="ExternalInput")
    wo_d = nc.dram_tensor("wo", [C, D], BF16, kind="ExternalInput")
    mb_d = nc.dram_tensor("mb", [128, nblk], F32, kind="ExternalInput")
    out_d = nc.dram_tensor("out", [T, D], F32, kind="ExternalOutput")

    with tile.TileContext(nc) as tc:
        with (
            tc.tile_pool(name="persist", bufs=1) as persist,
            tc.tile_pool(name="pt", bufs=4) as ppool,
            tc.tile_pool(name="rs", bufs=4) as rpool,
            tc.tile_pool(name="ostage", bufs=4) as ostage,
        ):
            # ---- persistent SBUF tensors + input DMA ----
            hT = persist.tile([128, KB, T], BF16)
            hTkv = persist.tile([128, KB, tkv], BF16)
            wq = persist.tile([128, KB, C], BF16)
            wk = persist.tile([128, KB, C], BF16)
            wv = persist.tile([128, KB, C], BF16)
            wo = persist.tile([128, C // 128, D], BF16)
            mb = persist.tile([128, nblk], F32)
            qT = persist.tile([128, C // 128, T], BF16)
            kT = persist.tile([128, C // 128, tkv], BF16)
            vsb = persist.tile([128, nblk, C], BF16)
            ocatT = persist.tile([128, C // 128, T], BF16)
            ones64 = persist.tile([128, 64], BF16)

            nc.sync.dma_start(wq[:], wq_d.ap().rearrange("(k p) c -> p k c", p=128))
            nc.sync.dma_start(hT[:], hT_d.ap().rearrange("(k p) t -> p k t", p=128))
            nc.sync.dma_start(hTkv[:], hTkv_d.ap().rearrange("(k p) t -> p k t", p=128))
            nc.sync.dma_start(wk[:], wk_d.ap().rearrange("(k p) c -> p k c", p=128))
            nc.sync.dma_start(wv[:], wv_d.ap().rearrange("(k p) c -> p k c", p=128))
            nc.sync.dma_start(wo[:], wo_d.ap().rearrange("(k p) e -> p k e", p=128))
            nc.sync.dma_start(mb[:], mb_d.ap())
            nc.vector.memset(ones64[:], 1.0)

            # ---- phase 1: QKV projections ----
            with tc.tile_pool(name="qkv_ps", bufs=4, space="PSUM") as qps:
                # Q^T [C, T]: lhsT = wq [D, C], rhs = hT [D, T]
                for cb in range(C // 128):
                    for tch in range(T // 512):
                        ps = qps.tile([128, 512], F32, tag="qkv")
                        for k in range(KB):
                            nc.tensor.matmul(
                                ps[:],
                                wq[:, k, cb * 128:(cb + 1) * 128],
                                hT[:, k, tch * 512:(tch + 1) * 512],
                                start=(k == 0), stop=(k == KB - 1),
                            )
                        nc.vector.tensor_copy(qT[:, cb, tch * 512:(tch + 1) * 512], ps[:])
                # K^T [C, tkv]
                for cb in range(C // 128):
                    for t0 in range(0, tkv, 512):
                        w = min(512, tkv - t0)
                        ps = qps.tile([128, 512], F32, tag="qkv")
                        for k in range(KB):
                            nc.tensor.matmul(
                                ps[:, :w],
                                wk[:, k, cb * 128:(cb + 1) * 128],
                                hTkv[:, k, t0:t0 + w],
                                start=(k == 0), stop=(k == KB - 1),
                            )
                        nc.vector.tensor_copy(kT[:, cb, t0:t0 + w], ps[:, :w])
                # V [tkv, C]
                for tb in range(nblk):
                    ps = qps.tile([128, 512], F32, tag="qkv")
                    for k in range(KB):
                        nc.tensor.matmul(
                            ps[:],
                            hTkv[:, k, tb * 128:(tb + 1) * 128],
                            wv[:, k, :],
                            start=(k == 0), stop=(k == KB - 1),
                        )
                    nc.vector.tensor_copy(vsb[:, tb, :], ps[:])

            # ---- phase 2: attention, per head pair (cb) and t1 half ----
            with (
                tc.tile_pool(name="s_ps", bufs=2, space="PSUM") as spool,
                tc.tile_pool(name="o_ps", bufs=2, space="PSUM") as opool,
                tc.tile_pool(name="r_ps", bufs=2, space="PSUM") as rps,
            ):
                for cb in range(C // 128):
                    hA, hB = 2 * cb, 2 * cb + 1
                    for half in range(2):
                        t1o = half * 1024
                        ops = [opool.tile([128, 512], F32, tag="o", name=f"o_{cb}_{half}_{i}")
                               for i in range(2)]
                        rss = [rps.tile([128, 512], F32, tag="r", name=f"r_{cb}_{half}_{i}")
                               for i in range(2)]
                        for blk in range(nblk):
                            sa = spool.tile([128, 1024], F32, tag="s")
                            sb_t = spool.tile([128, 1024], F32, tag="s")
                            for c2 in range(2):
                                t1s = slice(t1o + c2 * 512, t1o + (c2 + 1) * 512)
                                nc.tensor.matmul(
                                    sa[:, c2 * 512:(c2 + 1) * 512],
                                    kT[0:64, cb, blk * 128:(blk + 1) * 128],
                                    qT[0:64, cb, t1s],
                                    start=True, stop=True,
                                )
                                nc.tensor.matmul(
                                    sb_t[:, c2 * 512:(c2 + 1) * 512],
                                    kT[64:128, cb, blk * 128:(blk + 1) * 128],
                                    qT[64:128, cb, t1s],
                                    start=True, stop=True,
                                )
                            pta = ppool.tile([128, 1024], BF16, tag="pt")
                            nc.scalar.activation(
                                pta[:], sa[:], mybir.ActivationFunctionType.Exp,
                                bias=mb[:, blk:blk + 1], scale=SCALE,
                            )
                            ptb = ppool.tile([128, 1024], BF16, tag="pt")
                            nc.scalar.activation(
                                ptb[:], sb_t[:], mybir.ActivationFunctionType.Exp,
                                bias=mb[:, blk:blk + 1], scale=SCALE,
                            )
                            st, sp = (blk == 0), (blk == nblk - 1)
                            for c2 in range(2):
                                c2s = slice(c2 * 512, (c2 + 1) * 512)
                                # col-tiled pair: head A -> rows 0:64, head B -> rows 64:128
                                # of one bank; pending-zero is per-partition, so each half
                                # starts its own accumulation group at blk 0.
                                nc.tensor.matmul(
                                    ops[c2][0:64, :], vsb[:, blk, hA * 64:hA * 64 + 64],
                                    pta[:, c2s], start=st, stop=sp,
                                    tile_position=(0, 0), skip_group_check=True,
                                )
                                nc.tensor.matmul(
                                    ops[c2][64:128, :], vsb[:, blk, hB * 64:hB * 64 + 64],
                                    ptb[:, c2s], start=st, stop=sp,
                                    tile_position=(0, 64), skip_group_check=True,
                                )
                                nc.tensor.matmul(
                                    rss[c2][0:64, :], ones64[:],
                                    pta[:, c2s], start=st, stop=sp,
                                    tile_position=(0, 0), skip_group_check=True,
                                )
                                nc.tensor.matmul(
                                    rss[c2][64:128, :], ones64[:],
                                    ptb[:, c2s], start=st, stop=sp,
                                    tile_position=(0, 64), skip_group_check=True,
                                )
                        # normalize both heads at once: denom replicas are partition-aligned
                        for c2 in range(2):
                            t1s = slice(t1o + c2 * 512, t1o + (c2 + 1) * 512)
                            rsb = rpool.tile([128, 512], F32, tag="rs")
                            nc.vector.reciprocal_approx_fast(rsb[:], rss[c2][:])
                            nc.vector.tensor_mul(ocatT[:, cb, t1s], ops[c2][:], rsb[:])

            # ---- phase 3: output projection (partial; host adds pair + bias) ----
            with tc.tile_pool(name="proj_ps", bufs=4, space="PSUM") as jps:
                for tt in range(T // 128):
                    for e in range(D // 512):
                        ps = jps.tile([128, 512], F32, tag="proj")
                        for cb in range(C // 128):
                            nc.tensor.matmul(
                                ps[:],
                                ocatT[:, cb, tt * 128:(tt + 1) * 128],
                                wo[:, cb, e * 512:(e + 1) * 512],
                                start=(cb == 0), stop=(cb == C // 128 - 1),
                            )
                        ot = ostage.tile([128, 512], F32, tag="os")
                        nc.vector.tensor_copy(ot[:], ps[:])
                        nc.sync.dma_start(
                            out_d.ap()[tt * 128:(tt + 1) * 128, e * 512:(e + 1) * 512],
                            ot[:],
                        )

    nc.compile()
    return nc


_NC_CACHE = {}


def _get_nc(tkv=TKV):
    if tkv not in _NC_CACHE:
        _NC_CACHE[tkv] = build_bass(tkv)
    return _NC_CACHE[tkv]


def make_in_maps(inputs, tkv=TKV):
    hidden = np.asarray(inputs["hidden_states"], np.float32)
    mask = np.asarray(inputs["attention_mask"])
    Wq = np.asarray(inputs["Wq"], np.float32).astype(ml_dtypes.bfloat16)
    Wk = np.asarray(inputs["Wk"], np.float32).astype(ml_dtypes.bfloat16)
    Wv = np.asarray(inputs["Wv"], np.float32).astype(ml_dtypes.bfloat16)
    Wo = np.asarray(inputs["Wo"], np.float32).astype(ml_dtypes.bfloat16)

    nblk = tkv // 128
    in_maps = []
    for core in range(8):
        b, hh = divmod(core, 2)
        ch = slice(hh * C, (hh + 1) * C)
        hTb = np.ascontiguousarray(hidden[b].T).astype(ml_dtypes.bfloat16)
        idx = np.nonzero(mask[b])[0]
        n = len(idx)
        assert n <= tkv, f"unmasked keys {n} > TKV {tkv}"
        hTkv = np.zeros((D, tkv), ml_dtypes.bfloat16)
        hTkv[:, :n] = hidden[b].T[:, idx].astype(ml_dtypes.bfloat16)
        mbv = np.full(tkv, NEG, np.float32)
        mbv[:n] = 0.0
        mbv = np.ascontiguousarray(mbv.reshape(nblk, 128).T)  # [128, nblk]
        in_maps.append({
            "hT": hTb,
            "hTkv": hTkv,
            "wq": np.ascontiguousarray(Wq[:, ch]),
            "wk": np.ascontiguousarray(Wk[:, ch]),
            "wv": np.ascontiguousarray(Wv[:, ch]),
            "wo": np.ascontiguousarray(Wo[ch, :]),
            "mb": mbv,
        })
    return in_maps


def gather(results, inputs):
    bo = np.asarray(inputs["bo"], np.float32)
    out = np.empty((B, T, D), np.float32)
    for b in range(B):
        out[b] = results[2 * b]["out"] + results[2 * b + 1]["out"] + bo
    return out


def _pick_tkv(inputs):
    mask = np.asarray(inputs["attention_mask"])
    nmax = int(mask.sum(axis=1).max())
    return max(TKV, -(-nmax // 128) * 128)


def _run(inputs, trace=False):
    tkv = _pick_tkv(inputs)
    nc = _get_nc(tkv)
    in_maps = make_in_maps(inputs, tkv)
    res = run_bass_kernel_spmd(nc, in_maps, core_ids=list(range(8)), trace=trace)
    return gather(res.results, inputs), res


def kernel(**inputs):
    out, _ = _run(inputs)
    return out
